# revision 1
# baseline (speedup 1.0000x reference)
"""Trainium2 Bass kernel for nn_LossMeanCov (softmax filling + argmin segment mean/cov loss).

Self-contained: hardcodes shapes N=131072, D=32, K=64, 8 cores.

Strategy (data-parallel over N, 16384 points/core):
  Kernel 1 (per core): distances g = cc - 2 x.c via one fp16 matmul per
    128-point tile ([points, K] layout); DVE segmented min -> m; DVE
    broadcast-subtract h = g - m; ACT exp -> E (bf16); DVE segmented sum
    -> s; reciprocal -> r; PE matmul with r as weights accumulates the
    soft-filling partial sums in PSUM; gpsimd is_equal(h, 0) emits the
    one-hot argmin matrix (uint8) for the host.
  Host: pred = argmax(one-hot); builds a cluster-sorted, 128-padded,
    tile-major layout of x (pure data movement).
  Kernel 2 (per core): per-cluster second moments + sums as fp32 matmuls
    X'^T [X' | 1] accumulated into per-cluster PSUM windows (4-way
    column-tiled across PE col-groups).
  Host: sums partials over cores, forms means/covs, computes scalar loss.
"""

import sys
import numpy as np

sys.path.insert(0, "/opt/trn_rl_repo")

N, D, K = 131072, 32, 64
NCORES = 8
NLOC = N // NCORES          # 16384 points per core
NT = NLOC // 128            # 128 tiles of 128 points
BATCH = 8                   # tiles per processing batch
NB = NT // BATCH            # 16 batches
BETA = 10.0
KAPPA = 1.0

_CACHE = {}


def _bass_mods():
    import concourse.bacc as bacc
    import concourse.mybir as mybir
    from concourse.tile import TileContext
    from concourse.bass_utils import run_bass_kernel_spmd
    return bacc, mybir, TileContext, run_bass_kernel_spmd


def _build_k1(loop=1):
    bacc, mybir, TileContext, _ = _bass_mods()
    nc = bacc.Bacc("TRN2", target_bir_lowering=False)
    # rows 0..31: x^T (fp16), rows 32,33: ones (for the cc hi/lo pair)
    xt = nc.dram_tensor("xt", [34, NLOC], mybir.dt.float16, kind="ExternalInput")
    # rows 0..31: -2 c^T (fp16), row 32: cc_hi, row 33: cc_lo
    caug = nc.dram_tensor("caug", [34, K], mybir.dt.float16, kind="ExternalInput")
    a_out = nc.dram_tensor("a_out", [128, NT * K], mybir.dt.uint8, kind="ExternalOutput")
    fill_out = nc.dram_tensor("fill_out", [1, K], mybir.dt.float32, kind="ExternalOutput")

    with TileContext(nc) as tc:
        with tc.tile_pool(name="const", bufs=1) as constp, \
             tc.tile_pool(name="xtp", bufs=3) as xtp, \
             tc.tile_pool(name="gp", bufs=3, space="PSUM") as gp, \
             tc.tile_pool(name="fillp", bufs=1, space="PSUM") as fillp, \
             tc.tile_pool(name="hb", bufs=3) as hb, \
             tc.tile_pool(name="eb", bufs=3) as eb, \
             tc.tile_pool(name="ab", bufs=3) as ab, \
             tc.tile_pool(name="small", bufs=4) as smallp:
            c_t = constp.tile([34, K], mybir.dt.float16)
            nc.sync.dma_start(out=c_t[:], in_=caug[:])
            fill_ps = fillp.tile([1, K], mybir.dt.float32)

            def one_pass(_i=None):
                for b in range(NB):
                    xt_t = xtp.tile([34, BATCH * 128], mybir.dt.float16,
                                    tag="xt_t", name="xt_t")
                    nc.sync.dma_start(
                        out=xt_t[:], in_=xt[:, b * BATCH * 128:(b + 1) * BATCH * 128])
                    g_ps = gp.tile([128, BATCH * K], mybir.dt.float32,
                                   tag="g_ps", name="g_ps")
                    for t in range(BATCH):
                        nc.tensor.matmul(
                            g_ps[:, t * K:(t + 1) * K],
                            lhsT=xt_t[:, t * 128:(t + 1) * 128],
                            rhs=c_t[:],
                            start=True, stop=True)
                    g3 = g_ps[:].rearrange("p (t k) -> p t k", k=K)
                    m_t = smallp.tile([128, BATCH], mybir.dt.float32, tag="m", name="m_t")
                    nc.vector.tensor_reduce(
                        m_t[:], g3, axis=mybir.AxisListType.X, op=mybir.AluOpType.min)
                    h_t = hb.tile([128, BATCH * K], mybir.dt.float32,
                                  tag="h_t", name="h_t")
                    mb = m_t[:].unsqueeze(2).broadcast_to([128, BATCH, K])
                    nc.vector.tensor_tensor(
                        out=h_t[:].rearrange("p (t k) -> p t k", k=K),
                        in0=g3, in1=mb, op=mybir.AluOpType.subtract)
                    e_t = eb.tile([128, BATCH * K], mybir.dt.bfloat16,
                                  tag="e_t", name="e_t")
                    nc.scalar.activation(
                        e_t[:], h_t[:], mybir.ActivationFunctionType.Exp, scale=-BETA)
                    s_t = smallp.tile([128, BATCH], mybir.dt.float32, tag="s", name="s_t")
                    nc.vector.tensor_reduce(
                        s_t[:], e_t[:].rearrange("p (t k) -> p t k", k=K),
                        axis=mybir.AxisListType.X, op=mybir.AluOpType.add)
                    r_t = smallp.tile([128, BATCH], mybir.dt.float32, tag="r", name="r_t")
                    nc.vector.reciprocal(r_t[:], s_t[:])
                    r16 = smallp.tile([128, BATCH], mybir.dt.bfloat16, tag="r16", name="r16")
                    nc.vector.tensor_copy(r16[:], r_t[:])
                    for t in range(BATCH):
                        nc.tensor.matmul(
                            fill_ps[:],
                            lhsT=r16[:, t:t + 1],
                            rhs=e_t[:, t * K:(t + 1) * K],
                            start=(b == 0 and t == 0),
                            stop=(b == NB - 1 and t == BATCH - 1),
                            skip_group_check=True)
                    a_t = ab.tile([128, BATCH * K], mybir.dt.uint8, tag="a_t", name="a_t")
                    nc.gpsimd.tensor_scalar(
                        out=a_t[:], in0=h_t[:], scalar1=0.0, scalar2=None,
                        op0=mybir.AluOpType.is_equal)
                    nc.sync.dma_start(
                        out=a_out[:, b * BATCH * K:(b + 1) * BATCH * K], in_=a_t[:])

            if loop == 1:
                one_pass()
            else:
                with tc.For_i(0, loop, 1) as i:
                    one_pass(i)

            fill_sb = smallp.tile([1, K], mybir.dt.float32, tag="fill")
            nc.scalar.copy(fill_sb[:], fill_ps[:])
            nc.sync.dma_start(out=fill_out[:], in_=fill_sb[:])
    nc.compile()
    return nc


def _build_k2(caps, loop=1):
    """caps: tuple of 64 ints (multiples of 128) — per-cluster row capacity."""
    bacc, mybir, TileContext, _ = _bass_mods()
    ntiles = [c // 128 for c in caps]
    total_tiles = sum(ntiles)
    nc = bacc.Bacc("TRN2", target_bir_lowering=False)
    # tile-major sorted/padded points: [total_tiles, 128, 33]
    # col 32 is 1.0 for real rows, 0.0 for padding.
    fw = -(-total_tiles // 32)          # free windows per (bank, strip)
    assert fw * 33 <= 512
    xs = nc.dram_tensor("xs", [128, total_tiles, 33], mybir.dt.float32,
                        kind="ExternalInput")
    mom = nc.dram_tensor("mom", [8, 128, fw * 33], mybir.dt.float32,
                         kind="ExternalOutput")

    with TileContext(nc) as tc:
        with tc.tile_pool(name="xsp", bufs=6) as xsp, \
             tc.tile_pool(name="accp", bufs=1, space="PSUM") as accp, \
             tc.tile_pool(name="outp", bufs=2) as outp:
            acc = [accp.tile([128, fw * 33], mybir.dt.float32,
                             tag=f"acc{i}", name=f"acc{i}") for i in range(8)]
            for _ in range(loop):
                t0 = 0
                w = 0
                for k in range(K):
                    nt = ntiles[k]
                    if nt == 0:
                        continue
                    xk = xsp.tile([128, nt * 33], mybir.dt.float32,
                                  tag="xk", name="xk")
                    nc.sync.dma_start(
                        out=xk[:], in_=xs[:, t0:t0 + nt, :])
                    for j in range(nt):
                        strip = w % 4
                        bank = (w // 4) % 8
                        f = w // 32
                        nc.tensor.matmul(
                            acc[bank][32 * strip:32 * (strip + 1),
                                      33 * f:33 * f + 33],
                            lhsT=xk[:, j * 33:j * 33 + 32],
                            rhs=xk[:, j * 33:(j + 1) * 33],
                            start=True, stop=True,
                            tile_position=(0, 32 * strip))
                        w += 1
                    t0 += nt
            for i in range(8):
                ob = outp.tile([128, fw * 33], mybir.dt.float32, tag="ob", name="ob")
                nc.scalar.copy(ob[:], acc[i][:])
                nc.sync.dma_start(out=mom[i], in_=ob[:])
    nc.compile()
    return nc


def _get_k1():
    if "k1" not in _CACHE:
        _CACHE["k1"] = _build_k1()
    return _CACHE["k1"]


def _get_k2(caps):
    key = ("k2", caps)
    if key not in _CACHE:
        _CACHE[key] = _build_k2(caps)
    return _CACHE[key]


def _run(nc, in_maps, trace=False):
    *_, run_bass_kernel_spmd = _bass_mods()
    return run_bass_kernel_spmd(nc, in_maps, core_ids=list(range(NCORES)),
                                trace=trace)


_LAST_TIMES = {}


def kernel(x, cluster_centers, filling_target, means_target, covs_target,
           _trace=False):
    x = np.asarray(x, dtype=np.float32)
    c = np.asarray(cluster_centers, dtype=np.float32)
    filling_target = np.asarray(filling_target, dtype=np.float32)
    means_target = np.asarray(means_target, dtype=np.float32)
    covs_target = np.asarray(covs_target, dtype=np.float32)

    # ---- host prep for kernel 1 ----
    cc = (c * c).sum(1)                       # [K]
    cch = cc.astype(np.float16)
    ccl = (cc - cch.astype(np.float32)).astype(np.float16)
    caug = np.concatenate(
        [(-2.0 * c.T).astype(np.float16), cch[None, :], ccl[None, :]], axis=0)

    shards = x.reshape(NCORES, NLOC, D)
    in_maps1 = []
    ones2 = np.ones((2, NLOC), dtype=np.float16)
    for s in range(NCORES):
        xt = np.concatenate([shards[s].T.astype(np.float16), ones2], axis=0)
        in_maps1.append({"xt": np.ascontiguousarray(xt), "caug": caug})

    r1 = _run(_get_k1(), in_maps1, trace=_trace)
    _LAST_TIMES["k1"] = r1.exec_time_ns

    # ---- host: pred, counts, fill ----
    fill_sum = np.zeros(K, dtype=np.float64)
    preds = np.empty((NCORES, NLOC), dtype=np.int64)
    for s in range(NCORES):
        A = r1.results[s]["a_out"].reshape(128, NT, K)
        # point i = t*128 + p  ->  A[p, t, :]
        pred_pt = A.argmax(axis=2)            # [128(p), NT(t)]
        preds[s] = pred_pt.T.reshape(NLOC)
        fill_sum += r1.results[s]["fill_out"][0].astype(np.float64)
    filling = (fill_sum / N).astype(np.float32)
    loss_fil = np.mean((filling - filling_target) ** 2)

    counts_pc = np.zeros((NCORES, K), dtype=np.int64)
    for s in range(NCORES):
        counts_pc[s] = np.bincount(preds[s], minlength=K)
    counts = counts_pc.sum(0)

    caps = tuple(int(max(1, -(-int(counts_pc[:, k].max()) // 128)) * 128)
                 for k in range(K))

    # ---- host prep for kernel 2: cluster-sorted padded tile-major layout ----
    ntiles = [cp // 128 for cp in caps]
    total_tiles = sum(ntiles)
    offs = np.concatenate([[0], np.cumsum(caps)])[:K]
    in_maps2 = []
    for s in range(NCORES):
        xs = np.zeros((total_tiles * 128, 33), dtype=np.float32)
        pred = preds[s]
        order = np.argsort(pred, kind="stable")
        sorted_pred = pred[order]
        starts = np.concatenate([[0], np.cumsum(counts_pc[s])])[:K]
        within = np.arange(NLOC) - starts[sorted_pred]
        dest = offs[sorted_pred] + within
        xs[dest, :D] = shards[s][order]
        xs[dest, D] = 1.0
        xs_pm = np.ascontiguousarray(
            xs.reshape(total_tiles, 128, 33).transpose(1, 0, 2))
        in_maps2.append({"xs": xs_pm})

    r2 = _run(_get_k2(caps), in_maps2, trace=_trace)
    _LAST_TIMES["k2"] = r2.exec_time_ns

    # ---- host: combine moments, compute loss ----
    m2 = np.zeros((K, D, D), dtype=np.float64)
    sums = np.zeros((K, D), dtype=np.float64)
    tile_cluster = np.repeat(np.arange(K), ntiles)
    for s in range(NCORES):
        mom = r2.results[s]["mom"]            # [8, 128, fw*33]
        for w in range(total_tiles):
            k = tile_cluster[w]
            strip = w % 4
            bank = (w // 4) % 8
            f = w // 32
            W = mom[bank][32 * strip:32 * (strip + 1), 33 * f:33 * f + 33]
            m2[k] += W[:, :D]
            sums[k] += W[:, D]

    denom = np.maximum(counts.astype(np.float64), 1.0)
    means = sums / denom[:, None]
    covs = m2 / denom[:, None, None] - means[:, :, None] * means[:, None, :]
    loss_stat = np.mean((means - means_target.astype(np.float64)) ** 2) \
        + np.mean((covs - covs_target.astype(np.float64)) ** 2)
    total = loss_fil + KAPPA * loss_stat
    return np.float32(total)

